# revision 21
# baseline (speedup 1.0000x reference)
"""AngularAttention (windowed cosine attention) Trainium2 kernel, 8-core data-parallel.

Per core = 2 images = 32 windows x 196 tokens.
V4: batched x DMA; A2 chain balanced over scalar/DVE/gpsimd; V-projection (A3)
moved into the per-window B loop; B psum fully double-buffered in 8 banks
(S tiles are 2-bank with same-bank hp pairs serialized via explicit deps —
concurrent row-tiled matmuls must drain into distinct PSUM banks).
"""
import sys

sys.path.insert(0, "/opt/trn_rl_repo")

import numpy as np

import concourse.bacc as bacc
import concourse.mybir as mybir
import concourse.tile as tile
from concourse.tile import add_dep_helper
from concourse.bass_utils import run_bass_kernel_spmd

F32 = mybir.dt.float32
F32R = mybir.dt.float32r
BF16 = mybir.dt.bfloat16
AF = mybir.ActivationFunctionType
MUL = mybir.AluOpType.mult
ADD = mybir.AluOpType.add

NCORES = 8
NW = 32
L = 196
T = NW * L
E = 256
SCALE = 10.0

CHUNKS = [(i * 512, 512) for i in range(12)] + [(6144, 128)]
KC = [(0, 128), (128, 68)]


def _build():
    nc = bacc.Bacc(None)
    x = nc.declare_dram_parameter("x", [NW, L, E], F32, isOutput=False)
    w_qkv = nc.declare_dram_parameter("w_qkv", [2, 128, 768], F32R, isOutput=False)
    bqkT = nc.declare_dram_parameter("bqkT", [128, 4], F32, isOutput=False)
    w_proj = nc.declare_dram_parameter("w_proj", [2, 128, 256], F32R, isOutput=False)
    bv_bc = nc.declare_dram_parameter("bv_bc", [128, 256], F32, isOutput=False)
    bp_bc = nc.declare_dram_parameter("bp_bc", [128, 2, 256], F32, isOutput=False)
    identr = nc.declare_dram_parameter("identr", [128, 128], F32, isOutput=False)
    ind16 = nc.declare_dram_parameter("ind16", [128, 4, 128], F32R, isOutput=False)
    bcT = nc.declare_dram_parameter("bcT", [128, 4, 128], F32R, isOutput=False)
    onesb = nc.declare_dram_parameter("onesb", [128, 32], BF16, isOutput=False)
    out = nc.declare_dram_parameter("out", [NW, L, E], F32, isOutput=True)

    with tile.TileContext(nc) as tc:
        from contextlib import ExitStack

        with ExitStack() as root:
            const = root.enter_context(tc.tile_pool(name="const", bufs=1))
            sb_m = const.tile([128, 772], F32)
            nc.sync.dma_start(out=sb_m[:, 0:4], in_=bqkT[:, :])
            nc.sync.dma_start(out=sb_m[:, 4:260], in_=bv_bc[:, :])
            nc.sync.dma_start(
                out=sb_m[:, 260:772], in_=bp_bc[:, :, :].rearrange("p a b -> p (a b)")
            )
            sb_bqkT = sb_m[:, 0:4]
            sb_bv = sb_m[:, 4:260]
            sb_bp = sb_m[:, 260:772].rearrange("p (a b) -> p a b", a=2)
            sb_r = const.tile([128, 3072], F32R)
            nc.sync.dma_start(
                out=sb_r[:, 0:1536].rearrange("p (c f) -> p c f", c=2),
                in_=w_qkv[:, :, :].rearrange("c p f -> p c f"),
            )
            nc.sync.dma_start(
                out=sb_r[:, 1536:2048], in_=ind16[:, :, :].rearrange("p a b -> p (a b)")
            )
            nc.sync.dma_start(
                out=sb_r[:, 2048:2560].rearrange("p (c f) -> p c f", c=2),
                in_=w_proj[:, :, :].rearrange("c p f -> p c f"),
            )
            nc.sync.dma_start(
                out=sb_r[:, 2560:3072], in_=bcT[:, :, :].rearrange("p a b -> p (a b)")
            )
            sb_wqkv = sb_r[:, 0:1536].rearrange("p (c f) -> p c f", c=2)
            sb_i16 = sb_r[:, 1536:2048].rearrange("p (a b) -> p a b", a=4)
            sb_wproj = sb_r[:, 2048:2560].rearrange("p (c f) -> p c f", c=2)
            sb_bcT = sb_r[:, 2560:3072].rearrange("p (a b) -> p a b", a=4)
            sb_idr_t = const.tile([128, 128], F32)
            nc.sync.dma_start(out=sb_idr_t[:], in_=identr[:, :])
            sb_id = sb_idr_t[:]
            sb_ones_t = const.tile([128, 32], BF16)
            nc.sync.dma_start(out=sb_ones_t[:], in_=onesb[:, :])
            sb_ones = sb_ones_t[:]

            big = root.enter_context(tc.tile_pool(name="big", bufs=1))
            qkN = big.tile([128, 4, T], BF16)    # q heads 0-3 | q 4-7 | k 0-3 | k 4-7
            V0 = big.tile([128, NW, 256], BF16)
            V1 = big.tile([68, NW, 256], BF16)
            Vt = {0: V0, 1: V1}
            Vv = {ci: Vt[ci][:].rearrange("p w (h d) -> p w h d", h=8) for ci in (0, 1)}
            xtp = root.enter_context(tc.tile_pool(name="xtp", bufs=1))
            xT = xtp.tile([128, 2, T], F32R)

            # ---------------- Stage A: x transpose + qkT + cosine norm ---------
            with ExitStack() as sa:
                xin = sa.enter_context(tc.tile_pool(name="xin", bufs=2))
                ptr = sa.enter_context(tc.tile_pool(name="ptr", bufs=3, space="PSUM"))
                pqk = sa.enter_context(tc.tile_pool(name="pqk", bufs=3, space="PSUM"))
                pnrm = sa.enter_context(tc.tile_pool(name="pnrm", bufs=2, space="PSUM"))
                rawp = sa.enter_context(tc.tile_pool(name="rawp", bufs=3))
                sqp = sa.enter_context(tc.tile_pool(name="sqp", bufs=3))
                nmp = sa.enter_context(tc.tile_pool(name="nmp", bufs=2))

                # A1: load x in 4-window blocks, PE-transpose into xT (f32r)
                for wb in range(NW // 4):
                    x_f = xin.tile([98, 4, 2, 256], F32, tag="xin", name="xin")
                    nc.sync.dma_start(
                        out=x_f[:],
                        in_=x[4 * wb : 4 * wb + 4, :, :].rearrange(
                            "w (th p) e -> p w th e", th=2
                        ),
                    )
                    for wl in range(4):
                        w = 4 * wb + wl
                        pt = ptr.tile([128, 4, 128], F32, tag="tr", name="tr")
                        for th in range(2):
                            for e in range(2):
                                nc.tensor.transpose(
                                    pt[:, 2 * th + e, 0:98],
                                    x_f[:, wl, th, 128 * e : 128 * e + 128],
                                    sb_id[0:98, 0:98],
                                )
                        pt4 = pt[:].rearrange("p (th e) f -> p th e f", th=2)
                        nc.scalar.activation(
                            xT[:, 0, w * L : w * L + L].rearrange(
                                "p (th f) -> p th f", th=2
                            ),
                            pt4[:, :, 0, 0:98],
                            AF.Copy,
                        )
                        nc.vector.tensor_copy(
                            xT[:, 1, w * L : w * L + L].rearrange(
                                "p (th f) -> p th f", th=2
                            ),
                            pt4[:, :, 1, 0:98],
                        )

                # A2: qkT + cosine normalization per token chunk
                SQ_ENG = {0: nc.gpsimd, 1: nc.gpsimd, 2: nc.vector, 3: nc.vector}
                for c0, cs in CHUNKS:
                    raw = rawp.tile([128, 4, 512], BF16, tag="raw", name="raw")
                    sq = sqp.tile([128, 4, 512], F32R, tag="sq", name="sq")
                    for half in range(2):
                        ss_ps = pnrm.tile([128, 512], F32, tag="nrm", name="ssps")
                        for ft in (2 * half, 2 * half + 1):
                            qk_ps = pqk.tile([128, 512], F32, tag="qk", name="qkps")
                            for e in range(2):
                                nc.tensor.matmul(
                                    qk_ps[:, 0:cs],
                                    sb_wqkv[:, e, 128 * ft : 128 * ft + 128],
                                    xT[:, e, c0 : c0 + cs],
                                    start=(e == 0),
                                    stop=(e == 1),
                                )
                            nc.scalar.activation(
                                raw[:, ft, 0:cs], qk_ps[:, 0:cs], AF.Identity,
                                bias=sb_bqkT[:, ft : ft + 1], scale=1.0,
                            )
                            SQ_ENG[ft].tensor_tensor(
                                out=sq[:, ft, 0:cs], in0=raw[:, ft, 0:cs],
                                in1=raw[:, ft, 0:cs], op=MUL,
                            )
                            nc.tensor.matmul(
                                ss_ps[:, 0:cs], sb_i16[:, ft, :], sq[:, ft, 0:cs],
                                start=(ft == 2 * half), stop=(ft == 2 * half + 1),
                            )
                        rss = nmp.tile([128, 512], F32, tag="rss", name="rss")
                        nc.vector.reciprocal_approx_fast(out=rss[:, 0:cs], in_=ss_ps[:, 0:cs])
                        invn = nmp.tile([128, 512], F32R, tag="invn", name="invn")
                        nc.scalar.activation(invn[:, 0:cs], rss[:, 0:cs], AF.Sqrt, scale=1.0)
                        for ft in (2 * half, 2 * half + 1):
                            bc_ps = pnrm.tile([128, 512], F32, tag="nrm", name="bcps")
                            nc.tensor.matmul(
                                bc_ps[:, 0:cs], sb_bcT[:, ft, :], invn[:, 0:cs],
                                start=True, stop=True,
                            )
                            nc.vector.tensor_tensor(
                                out=qkN[:, ft, c0 : c0 + cs], in0=raw[:, ft, 0:cs],
                                in1=bc_ps[:, 0:cs], op=MUL,
                            )

            # ---------------- Stage B: per-window pipeline ----------------
            with ExitStack() as sb_:
                pS = sb_.enter_context(tc.tile_pool(name="pS", bufs=1, space="PSUM"))
                pAB = sb_.enter_context(tc.tile_pool(name="pAB", bufs=1, space="PSUM"))
                pyv = sb_.enter_context(tc.tile_pool(name="pyv", bufs=3, space="PSUM"))
                eqp = sb_.enter_context(tc.tile_pool(name="eqp", bufs=3))
                invp = sb_.enter_context(tc.tile_pool(name="invp", bufs=2))
                etp = sb_.enter_context(tc.tile_pool(name="etp", bufs=2))
                ysb = sb_.enter_context(tc.tile_pool(name="ysb", bufs=3))

                for w in range(NW):
                    # A3 (moved here): V for window w
                    for ci, (k0, ks) in enumerate(KC):
                        v_ps = pyv.tile([128, 512], F32, tag="yv", name="vps")
                        vp = v_ps[:]
                        for e in range(2):
                            nc.tensor.matmul(
                                vp[0:ks, 0:256],
                                xT[:, e, w * L + k0 : w * L + k0 + ks],
                                sb_wqkv[:, e, 512:768],
                                start=(e == 0), stop=(e == 1),
                            )
                        nc.vector.scalar_tensor_tensor(
                            out=Vt[ci][:, w, :], in0=vp[0:ks, 0:256], scalar=1.0,
                            in1=sb_bv[0:ks, :], op0=MUL, op1=ADD,
                        )
                    # B1: S = Kn^T Qn; exp -> EQ
                    EQ = {
                        ci: eqp.tile([ks, 8, 196], BF16, tag=f"eq{ci}", name=f"eq{ci}")
                        for ci, (k0, ks) in enumerate(KC)
                    }
                    for g in range(2):
                        for ci, (k0, ks) in enumerate(KC):
                            sps = pS.tile([128, 4, 512], F32, tag="sps", name="sps")
                            for hp in range(4):
                                nc.tensor.matmul(
                                    sps[0:ks, hp, 0:L],
                                    qkN[32 * hp : 32 * hp + 32, 2 + g,
                                        w * L + k0 : w * L + k0 + ks],
                                    qkN[32 * hp : 32 * hp + 32, g, w * L : w * L + L],
                                    start=True, stop=True,
                                    tile_position=(32 * hp, 0),
                                )
                            nc.scalar.activation(
                                EQ[ci][0:ks, 4 * g : 4 * g + 4, :],
                                sps[0:ks, :, 0:L],
                                AF.Exp, scale=SCALE,
                            )
                    # B2: AV (slot 0) + denominator (slot 1); divide -> eT
                    eTs = []
                    for g in range(2):
                        pab = pAB.tile([128, 2, 256], F32, tag="pab", name="pab")
                        for hp in range(4):
                            h = 4 * g + hp
                            for ci, (k0, ks) in enumerate(KC):
                                nc.tensor.matmul(
                                    pab[32 * hp : 32 * hp + 32, 0, 0:L],
                                    Vv[ci][:, w, h, :],
                                    EQ[ci][:, h, :],
                                    start=(ci == 0), stop=(ci == 1),
                                    tile_position=(0, 32 * hp),
                                )
                        for hp in range(4):
                            h = 4 * g + hp
                            for ci, (k0, ks) in enumerate(KC):
                                nc.tensor.matmul(
                                    pab[32 * hp : 32 * hp + 32, 1, 0:L],
                                    sb_ones[0:ks, :],
                                    EQ[ci][:, h, :],
                                    start=(ci == 0), stop=(ci == 1),
                                    tile_position=(0, 32 * hp),
                                )
                        inv = invp.tile([128, 196], F32, tag="inv", name="inv")
                        nc.vector.reciprocal_approx_fast(out=inv[:], in_=pab[:, 1, 0:L])
                        eT = etp.tile([128, 196], F32R, tag=f"eT{g}", name=f"eT{g}")
                        eTs.append(eT)
                        with nc.allow_low_precision(reason="attn out f32r for proj"):
                            nc.vector.tensor_tensor(
                                out=eT[:], in0=pab[:, 0, 0:L], in1=inv[:], op=MUL
                            )
                    # proj
                    y_t = pyv.tile([128, 512], F32, tag="yv", name="yps")
                    y_ps = y_t[:].rearrange("p (a b) -> p a b", a=2)[0:98]
                    for th in range(2):
                        for g in range(2):
                            nc.tensor.matmul(
                                y_ps[:, th, :],
                                eTs[g][:, 98 * th : 98 * th + 98],
                                sb_wproj[:, g, :],
                                start=(g == 0), stop=(g == 1),
                            )
                    y_sb = ysb.tile([98, 2, 256], F32, tag="ysb", name="ysb")
                    nc.vector.scalar_tensor_tensor(
                        out=y_sb[:], in0=y_ps[:], scalar=1.0, in1=sb_bp[0:98, :, :],
                        op0=MUL, op1=ADD,
                    )
                    nc.sync.dma_start(
                        out=out[w, :, :].rearrange("(th p) e -> p th e", th=2),
                        in_=y_sb[:],
                    )

    nc.finalize()
    return nc


_NC = None


def _get_nc():
    global _NC
    if _NC is None:
        _NC = _build()
    return _NC


def _consts():
    import ml_dtypes

    p = np.arange(128)
    ind16 = np.full((128, 4, 128), 1e-6, np.float32)
    bcT = np.zeros((128, 4, 128), np.float32)
    for ft in range(4):
        ind16[p, ft, 4 * ft + p // 32] = 1.0
        bcT[4 * ft + p // 32, ft, p] = 1.0
    return {
        "identr": np.eye(128, dtype=np.float32),
        "ind16": ind16,
        "bcT": bcT,
        "onesb": np.ones((128, 32), ml_dtypes.bfloat16),
    }


def kernel(x, w_qkv, b_qkv, w_proj, b_proj, _trace=False):
    x = np.ascontiguousarray(np.asarray(x, np.float32))
    w_qkv = np.asarray(w_qkv, np.float32)
    b_qkv = np.asarray(b_qkv, np.float32)
    w_proj = np.asarray(w_proj, np.float32)
    b_proj = np.asarray(b_proj, np.float32)

    B, H, W, _ = x.shape
    xw = (
        x.reshape(NCORES, 2, 4, 14, 4, 14, E)
        .transpose(0, 1, 2, 4, 3, 5, 6)
        .reshape(NCORES, NW, L, E)
    )
    base = {
        "w_qkv": np.ascontiguousarray(w_qkv.reshape(2, 128, 768)),
        "bqkT": np.ascontiguousarray(b_qkv[:512].reshape(4, 128).T),
        "w_proj": np.ascontiguousarray(w_proj.reshape(2, 128, 256)),
        "bv_bc": np.broadcast_to(b_qkv[512:768], (128, 256)).copy(),
        "bp_bc": np.broadcast_to(b_proj, (128, 2, 256)).copy(),
        **_consts(),
    }
    in_maps = [dict(base, x=np.ascontiguousarray(xw[c])) for c in range(NCORES)]

    nc = _get_nc()
    br = run_bass_kernel_spmd(nc, in_maps, list(range(NCORES)), trace=_trace)
    outs = np.stack([br.results[c]["out"] for c in range(NCORES)])
    y = (
        outs.reshape(NCORES, 2, 4, 4, 14, 14, E)
        .transpose(0, 1, 2, 4, 3, 5, 6)
        .reshape(B, H, W, E)
    )
    if _trace:
        return y, br.exec_time_ns
    return y


# revision 22
# speedup vs baseline: 1.4019x; 1.4019x over previous
"""AngularAttention (windowed cosine attention) Trainium2 kernel, 8-core data-parallel.

Per core = 2 images = 32 windows x 196 tokens.
V2: batched x DMA, bf16 single-pass transposes (gpsimd casts), per-window
pipelined B-stage (S -> exp -> AV/den -> div -> proj) in 8 psum banks.
"""
import sys

sys.path.insert(0, "/opt/trn_rl_repo")

import numpy as np

import concourse.bacc as bacc
import concourse.mybir as mybir
import concourse.tile as tile
from concourse.bass_utils import run_bass_kernel_spmd

F32 = mybir.dt.float32
F32R = mybir.dt.float32r
BF16 = mybir.dt.bfloat16
AF = mybir.ActivationFunctionType
MUL = mybir.AluOpType.mult
ADD = mybir.AluOpType.add

NCORES = 8
NW = 32
L = 196
T = NW * L
E = 256
SCALE = 10.0

CHUNKS = [(i * 512, 512) for i in range(12)] + [(6144, 128)]
KC = [(0, 128), (128, 68)]


def _build():
    nc = bacc.Bacc(None)
    x = nc.declare_dram_parameter("x", [NW, L, E], F32, isOutput=False)
    w_qkv = nc.declare_dram_parameter("w_qkv", [2, 128, 768], F32R, isOutput=False)
    bqkT = nc.declare_dram_parameter("bqkT", [128, 4], F32, isOutput=False)
    w_proj = nc.declare_dram_parameter("w_proj", [2, 128, 256], F32R, isOutput=False)
    bv_bc = nc.declare_dram_parameter("bv_bc", [128, 256], F32, isOutput=False)
    bp_bc = nc.declare_dram_parameter("bp_bc", [128, 2, 256], F32, isOutput=False)
    identr = nc.declare_dram_parameter("identr", [128, 128], F32, isOutput=False)
    ind16 = nc.declare_dram_parameter("ind16", [128, 4, 128], F32R, isOutput=False)
    bcT = nc.declare_dram_parameter("bcT", [128, 4, 128], F32R, isOutput=False)
    onesb = nc.declare_dram_parameter("onesb", [128, 32], BF16, isOutput=False)
    out = nc.declare_dram_parameter("out", [NW, L, E], F32, isOutput=True)

    with tile.TileContext(nc) as tc:
        from contextlib import ExitStack

        with ExitStack() as root:
            const = root.enter_context(tc.tile_pool(name="const", bufs=1))
            # f32 arena: [0:4] bqkT, [4:260] bv, [260:772] bp
            sb_m = const.tile([128, 772], F32)
            nc.sync.dma_start(out=sb_m[:, 0:4], in_=bqkT[:, :])
            nc.sync.dma_start(out=sb_m[:, 4:260], in_=bv_bc[:, :])
            nc.sync.dma_start(
                out=sb_m[:, 260:772], in_=bp_bc[:, :, :].rearrange("p a b -> p (a b)")
            )
            sb_bqkT = sb_m[:, 0:4]
            sb_bv = sb_m[:, 4:260]
            sb_bp = sb_m[:, 260:772].rearrange("p (a b) -> p a b", a=2)
            # f32r arena: [0:1536] w_qkv (2,768), [1536:2048] ind16, [2048:2560] w_proj, [2560:3072] bcT
            sb_r = const.tile([128, 3072], F32R)
            nc.sync.dma_start(
                out=sb_r[:, 0:1536].rearrange("p (c f) -> p c f", c=2),
                in_=w_qkv[:, :, :].rearrange("c p f -> p c f"),
            )
            nc.sync.dma_start(
                out=sb_r[:, 1536:2048], in_=ind16[:, :, :].rearrange("p a b -> p (a b)")
            )
            nc.sync.dma_start(
                out=sb_r[:, 2048:2560].rearrange("p (c f) -> p c f", c=2),
                in_=w_proj[:, :, :].rearrange("c p f -> p c f"),
            )
            nc.sync.dma_start(
                out=sb_r[:, 2560:3072], in_=bcT[:, :, :].rearrange("p a b -> p (a b)")
            )
            sb_wqkv = sb_r[:, 0:1536].rearrange("p (c f) -> p c f", c=2)
            sb_i16 = sb_r[:, 1536:2048].rearrange("p (a b) -> p a b", a=4)
            sb_wproj = sb_r[:, 2048:2560].rearrange("p (c f) -> p c f", c=2)
            sb_bcT = sb_r[:, 2560:3072].rearrange("p (a b) -> p a b", a=4)
            # f32 ident for transposes; bf16 ones
            sb_idr_t = const.tile([128, 128], F32)
            nc.sync.dma_start(out=sb_idr_t[:], in_=identr[:, :])
            sb_id = sb_idr_t[:]
            sb_ones_t = const.tile([128, 32], BF16)
            nc.sync.dma_start(out=sb_ones_t[:], in_=onesb[:, :])
            sb_ones = sb_ones_t[:]

            big = root.enter_context(tc.tile_pool(name="big", bufs=1))
            qkN = big.tile([128, 4, T], BF16)    # q heads 0-3 | q 4-7 | k 0-3 | k 4-7
            V0 = big.tile([128, NW, 256], BF16)  # k-chunk0 of each window, [tok, (head d)]
            V1 = big.tile([68, NW, 256], BF16)
            Vt = {0: V0, 1: V1}
            Vv = {ci: Vt[ci][:].rearrange("p w (h d) -> p w h d", h=8) for ci in (0, 1)}

            # ---------------- Stage A ----------------
            with ExitStack() as sa:
                xtp = sa.enter_context(tc.tile_pool(name="xtp", bufs=1))
                xT = xtp.tile([128, 2, T], F32R)

                xin = sa.enter_context(tc.tile_pool(name="xin", bufs=2))
                ptr = sa.enter_context(tc.tile_pool(name="ptr", bufs=3, space="PSUM"))
                pqk = sa.enter_context(tc.tile_pool(name="pqk", bufs=3, space="PSUM"))
                pnrm = sa.enter_context(tc.tile_pool(name="pnrm", bufs=2, space="PSUM"))
                rawp = sa.enter_context(tc.tile_pool(name="rawp", bufs=3))
                sqp = sa.enter_context(tc.tile_pool(name="sqp", bufs=3))
                nmp = sa.enter_context(tc.tile_pool(name="nmp", bufs=2))

                # A1: load x in 4-window blocks, cast to bf16 (gpsimd), PE-transpose
                # A3: V natural, emitted per window right after its transposes
                for wb in range(NW // 4):
                    x_f = xin.tile([98, 4, 2, 256], F32, tag="xin", name="xin")
                    nc.sync.dma_start(
                        out=x_f[:],
                        in_=x[4 * wb : 4 * wb + 4, :, :].rearrange(
                            "w (th p) e -> p w th e", th=2
                        ),
                    )
                    for wl in range(4):
                        w = 4 * wb + wl
                        pt = ptr.tile([128, 4, 128], F32, tag="tr", name="tr")
                        for th in range(2):
                            for e in range(2):
                                nc.tensor.transpose(
                                    pt[:, 2 * th + e, 0:98],
                                    x_f[:, wl, th, 128 * e : 128 * e + 128],
                                    sb_id[0:98, 0:98],
                                )
                        pt4 = pt[:].rearrange("p (th e) f -> p th e f", th=2)
                        for e in range(2):
                            nc.vector.tensor_copy(
                                xT[:, e, w * L : w * L + L].rearrange(
                                    "p (th f) -> p th f", th=2
                                ),
                                pt4[:, :, e, 0:98],
                            )
                        # A3: V for window w
                        for ci, (k0, ks) in enumerate(KC):
                            v_ps = ptr.tile([128, 4, 128], F32, tag="tr", name="vps")
                            vp = v_ps[:].rearrange("p a b -> p (a b)")
                            for e in range(2):
                                nc.tensor.matmul(
                                    vp[0:ks, 0:256],
                                    xT[:, e, w * L + k0 : w * L + k0 + ks],
                                    sb_wqkv[:, e, 512:768],
                                    start=(e == 0),
                                    stop=(e == 1),
                                )
                            nc.vector.scalar_tensor_tensor(
                                out=Vt[ci][:, w, :], in0=vp[0:ks, 0:256], scalar=1.0,
                                in1=sb_bv[0:ks, :],
                                op0=MUL, op1=ADD,
                            )

                # A2: qkT + cosine normalization, per token chunk (forward order)
                for c0, cs in CHUNKS:
                    raw = rawp.tile([128, 4, 512], BF16, tag="raw", name="raw")
                    sq = sqp.tile([128, 4, 512], F32R, tag="sq", name="sq")
                    for half in range(2):
                        ss_ps = pnrm.tile([128, 512], F32, tag="nrm", name="ssps")
                        for ft in (2 * half, 2 * half + 1):
                            qk_ps = pqk.tile([128, 512], F32, tag="qk", name="qkps")
                            for e in range(2):
                                nc.tensor.matmul(
                                    qk_ps[:, 0:cs],
                                    sb_wqkv[:, e, 128 * ft : 128 * ft + 128],
                                    xT[:, e, c0 : c0 + cs],
                                    start=(e == 0),
                                    stop=(e == 1),
                                )
                            nc.scalar.activation(
                                raw[:, ft, 0:cs], qk_ps[:, 0:cs], AF.Identity,
                                bias=sb_bqkT[:, ft : ft + 1], scale=1.0,
                            )
                            sq_eng = nc.gpsimd if ft != 3 else nc.vector
                            sq_eng.tensor_tensor(
                                out=sq[:, ft, 0:cs], in0=raw[:, ft, 0:cs],
                                in1=raw[:, ft, 0:cs], op=MUL,
                            )
                            nc.tensor.matmul(
                                ss_ps[:, 0:cs], sb_i16[:, ft, :], sq[:, ft, 0:cs],
                                start=(ft == 2 * half), stop=(ft == 2 * half + 1),
                            )
                        rss = nmp.tile([128, 512], F32, tag="rss", name="rss")
                        nc.vector.reciprocal_approx_fast(out=rss[:, 0:cs], in_=ss_ps[:, 0:cs])
                        invn = nmp.tile([128, 512], F32R, tag="invn", name="invn")
                        nc.scalar.activation(invn[:, 0:cs], rss[:, 0:cs], AF.Sqrt, scale=1.0)
                        for ft in (2 * half, 2 * half + 1):
                            bc_ps = pnrm.tile([128, 512], F32, tag="nrm", name="bcps")
                            nc.tensor.matmul(
                                bc_ps[:, 0:cs], sb_bcT[:, ft, :], invn[:, 0:cs],
                                start=True, stop=True,
                            )
                            nc.vector.tensor_tensor(
                                out=qkN[:, ft, c0 : c0 + cs], in0=raw[:, ft, 0:cs],
                                in1=bc_ps[:, 0:cs], op=MUL,
                            )

            # ---------------- Stage B: per-window pipeline ----------------
            # All B psum rotates through one tag of [128,4,512] 4-bank tiles
            # (2 bufs = 8 banks): S(g0), S(g1), then AV/den/proj per window.
            # Row-tiled concurrent MMs land in distinct banks; ci1 S and proj
            # outputs use slot tails (no concurrent same-bank drains).
            with ExitStack() as sb_:
                pB_ = sb_.enter_context(tc.tile_pool(name="pB", bufs=2, space="PSUM"))
                eqp = sb_.enter_context(tc.tile_pool(name="eqp", bufs=3))
                invp = sb_.enter_context(tc.tile_pool(name="invp", bufs=2))
                etp = sb_.enter_context(tc.tile_pool(name="etp", bufs=2))
                ysb = sb_.enter_context(tc.tile_pool(name="ysb", bufs=3))
                CO = {0: 0, 1: 256}  # ci -> column offset inside a slot

                for w in range(NW):
                    EQ = {
                        ci: eqp.tile([ks, 8, 196], BF16, tag=f"eq{ci}", name=f"eq{ci}")
                        for ci, (k0, ks) in enumerate(KC)
                    }
                    # B1: S = Kn^T Qn per (ci, g); exp -> EQ
                    for g in range(2):
                        sps = pB_.tile([128, 4, 512], F32, tag="wps", name="sps")
                        for ci, (k0, ks) in enumerate(KC):
                            for hp in range(4):
                                nc.tensor.matmul(
                                    sps[0:ks, hp, CO[ci] : CO[ci] + L],
                                    qkN[32 * hp : 32 * hp + 32, 2 + g,
                                        w * L + k0 : w * L + k0 + ks],
                                    qkN[32 * hp : 32 * hp + 32, g, w * L : w * L + L],
                                    start=True, stop=True,
                                    tile_position=(32 * hp, 0),
                                )
                        for ci, (k0, ks) in enumerate(KC):
                            nc.scalar.activation(
                                EQ[ci][0:ks, 4 * g : 4 * g + 4, :],
                                sps[0:ks, :, CO[ci] : CO[ci] + L],
                                AF.Exp, scale=SCALE,
                            )
                    # B2: AV (slots 0,2) + denominator (slots 1,3); divide -> eT
                    aby = pB_.tile([128, 4, 512], F32, tag="wps", name="aby")
                    eTs = []
                    for g in range(2):
                        for hp in range(4):
                            h = 4 * g + hp
                            for ci, (k0, ks) in enumerate(KC):
                                nc.tensor.matmul(
                                    aby[32 * hp : 32 * hp + 32, 2 * g, 0:L],
                                    Vv[ci][:, w, h, :],
                                    EQ[ci][:, h, :],
                                    start=(ci == 0), stop=(ci == 1),
                                    tile_position=(0, 32 * hp),
                                )
                        for hp in range(4):
                            h = 4 * g + hp
                            for ci, (k0, ks) in enumerate(KC):
                                nc.tensor.matmul(
                                    aby[32 * hp : 32 * hp + 32, 2 * g + 1, 0:L],
                                    sb_ones[0:ks, :],
                                    EQ[ci][:, h, :],
                                    start=(ci == 0), stop=(ci == 1),
                                    tile_position=(0, 32 * hp),
                                )
                        inv = invp.tile([128, 196], F32, tag="inv", name="inv")
                        nc.vector.reciprocal_approx_fast(
                            out=inv[:], in_=aby[:, 2 * g + 1, 0:L]
                        )
                        eT = etp.tile([128, 196], F32R, tag=f"eT{g}", name=f"eT{g}")
                        eTs.append(eT)
                        with nc.allow_low_precision(reason="attn out f32r for proj"):
                            nc.vector.tensor_tensor(
                                out=eT[:], in0=aby[:, 2 * g, 0:L], in1=inv[:], op=MUL
                            )
                    # proj into slot tails of aby (serial after eT, no drain overlap)
                    for th in range(2):
                        for g in range(2):
                            nc.tensor.matmul(
                                aby[0:98, th, 256:512],
                                eTs[g][:, 98 * th : 98 * th + 98],
                                sb_wproj[:, g, :],
                                start=(g == 0), stop=(g == 1),
                            )
                    y_sb = ysb.tile([98, 2, 256], F32, tag="ysb", name="ysb")
                    nc.vector.scalar_tensor_tensor(
                        out=y_sb[:], in0=aby[0:98, 0:2, 256:512], scalar=1.0,
                        in1=sb_bp[0:98, :, :],
                        op0=MUL, op1=ADD,
                    )
                    nc.sync.dma_start(
                        out=out[w, :, :].rearrange("(th p) e -> p th e", th=2),
                        in_=y_sb[:],
                    )

    nc.finalize()
    return nc


_NC = None


def _get_nc():
    global _NC
    if _NC is None:
        _NC = _build()
    return _NC


def _consts():
    import ml_dtypes

    p = np.arange(128)
    ind16 = np.full((128, 4, 128), 1e-6, np.float32)
    bcT = np.zeros((128, 4, 128), np.float32)
    for ft in range(4):
        ind16[p, ft, 4 * ft + p // 32] = 1.0
        bcT[4 * ft + p // 32, ft, p] = 1.0
    return {
        "identr": np.eye(128, dtype=np.float32),
        "ind16": ind16,
        "bcT": bcT,
        "onesb": np.ones((128, 32), ml_dtypes.bfloat16),
    }


def kernel(x, w_qkv, b_qkv, w_proj, b_proj, _trace=False):
    x = np.ascontiguousarray(np.asarray(x, np.float32))
    w_qkv = np.asarray(w_qkv, np.float32)
    b_qkv = np.asarray(b_qkv, np.float32)
    w_proj = np.asarray(w_proj, np.float32)
    b_proj = np.asarray(b_proj, np.float32)

    B, H, W, _ = x.shape
    xw = (
        x.reshape(NCORES, 2, 4, 14, 4, 14, E)
        .transpose(0, 1, 2, 4, 3, 5, 6)
        .reshape(NCORES, NW, L, E)
    )
    base = {
        "w_qkv": np.ascontiguousarray(w_qkv.reshape(2, 128, 768)),
        "bqkT": np.ascontiguousarray(b_qkv[:512].reshape(4, 128).T),
        "w_proj": np.ascontiguousarray(w_proj.reshape(2, 128, 256)),
        "bv_bc": np.broadcast_to(b_qkv[512:768], (128, 256)).copy(),
        "bp_bc": np.broadcast_to(b_proj, (128, 2, 256)).copy(),
        **_consts(),
    }
    in_maps = [dict(base, x=np.ascontiguousarray(xw[c])) for c in range(NCORES)]

    nc = _get_nc()
    br = run_bass_kernel_spmd(nc, in_maps, list(range(NCORES)), trace=_trace)
    outs = np.stack([br.results[c]["out"] for c in range(NCORES)])
    y = (
        outs.reshape(NCORES, 2, 4, 4, 14, 14, E)
        .transpose(0, 1, 2, 4, 3, 5, 6)
        .reshape(B, H, W, E)
    )
    if _trace:
        return y, br.exec_time_ns
    return y
